# revision 11
# baseline (speedup 1.0000x reference)
"""GCN layer kernel for Trainium2 (8 NeuronCores, SPMD).

out = relu( D^{-1/2} (A+I) D^{-1/2} x W^T + b )

Math restructure (projection commutes with aggregation):
    out[i] = relu( dinv[i] * (sum_{(i,j) in E+self} xp[j]) @ W^T + b )
with xp = dinv[:,None] * x  (host-prescaled, bf16).

Bottleneck analysis (HW traces): SWDGE gather-descriptor emission on the
Pool/GpSimd engine runs at ~2.5 ns/row aggregate (4 queues, ucode max) and
is the critical path; DMA drain is ~27.5 ns per 512B descriptor over 16
engines (~1.7 ns/row).  So the kernel minimizes gathered ROWS and keeps
emission running continuously:

  - Edges are bucketed by (src chunk of 128, lo/hi int16 window) and packed
    into two core-uniform streams with per-(chunk,window) runs padded only
    to the max count over the 8 cores (not to 128): ~104.6k rows/core vs
    108k for per-chunk 128-padding.
  - Each window's stream is fetched by a handful of large dma_gathers
    (~40 blocks each) round-robined over the 4 SWDGE queues; blocks may
    span chunk boundaries, handled by per-segment one-hot S matrices.
  - Self-loop rows are a contiguous HWDGE load (no descriptors).
  - Segment-sum via one-hot matmuls in f-major psum; per-chunk projection
    with in-psum bias (sqrt(deg) x b outer product) and a fused
    relu(dinv * .) ACT epilogue.

Host does only sharding/layout work: edge bucketing, int16 index packing,
degree counting, scaling/casts.
"""

import sys

for _p in ("/opt/trn_rl_repo",):
    if _p not in sys.path:
        sys.path.insert(0, _p)

from contextlib import ExitStack

import ml_dtypes
import numpy as np

import concourse.bass as bass
import concourse.mybir as mybir
import concourse.tile as tile
from concourse import bacc
from concourse.bass_utils import run_bass_kernel_spmd

BF16 = ml_dtypes.bfloat16
FP8 = ml_dtypes.float8_e3m4
QS = 6.0  # global pre-quantization scale (dodges e3m4 subnormal floor)

N_NODES = 50000
N_EDGES = 800000
F = 256  # in_size == out_size == 256
N_CORES = 8
NPC = N_NODES // N_CORES  # 6250 nodes per core
SPLIT = 32768  # int16 index limit for dma_gather
CHUNKS = (NPC + 127) // 128  # 49 chunks of <=128 src nodes per core
NSELF = CHUNKS * 128  # 6272 padded self rows per core
OUT_GRP = 4  # output chunks per DRAM write
S_BATCH = 8  # one-hot blocks built per DVE instruction
N_QUEUES = 4  # SWDGE queues (ucode max)
GB = 8  # gather granularity: 1024 rows (low per-gather emission latency)


def _pack_idx_stream(vals_at, n_pos):
    """int16 gather index layout: stream position i -> [i % 16, i // 16],
    replicated to 128 partitions. vals_at: (positions, values). Pad is 0
    (gathers row 0, masked to zero by the one-hot S whose pad slot is 200)."""
    n = ((n_pos + 127) // 128) * 128
    a = np.zeros(n, dtype=np.int16)
    pos, vals = vals_at
    a[pos] = vals
    cols = a.reshape(n // 16, 16).T  # [16, n/16]
    return np.tile(cols, (8, 1))  # [128, n/16]


def _segments(run, n_pos):
    """Split each chunk's [off, off+run) range at 128-block boundaries.
    Returns segs[k] = list of (block, p0, p1, off) in stream order."""
    segs = [[] for _ in range(CHUNKS)]
    off = 0
    for k in range(CHUNKS):
        a, b = off, off + int(run[k])
        p = a
        while p < b:
            blk = p // 128
            q = min(b, (blk + 1) * 128)
            segs[k].append((blk, p - blk * 128, q - blk * 128, p))
            p = q
        off = b
    assert off == n_pos
    return segs


def _build_program(run_lo, run_hi):
    """Build the (core-uniform) Bass program from per-(chunk,window) padded
    run lengths."""
    nc = bacc.Bacc(
        None, target_bir_lowering=False, debug=False, num_swdge_queues=N_QUEUES
    )
    dt = mybir.dt

    tot_lo = int(run_lo.sum())
    tot_hi = int(run_hi.sum())
    blk_lo = (tot_lo + 127) // 128
    blk_hi = (tot_hi + 127) // 128
    segs_lo = _segments(run_lo, tot_lo)
    segs_hi = _segments(run_hi, tot_hi)
    # consumption-order segment index (chunk-major, lo before hi)
    n_segs = sum(len(segs_lo[k]) + len(segs_hi[k]) for k in range(CHUNKS))

    xp = nc.dram_tensor("xp", [N_NODES, F], dt.float8e3, kind="ExternalInput")
    xself = nc.dram_tensor("xself", [NSELF, F], dt.bfloat16, kind="ExternalInput")
    wT = nc.dram_tensor("wt", [2, 128, F], dt.bfloat16, kind="ExternalInput")
    iota8 = nc.dram_tensor(
        "iota8", [128, S_BATCH, 128], dt.bfloat16, kind="ExternalInput"
    )
    ident = nc.dram_tensor("ident", [128, 128], dt.bfloat16, kind="ExternalInput")
    bias_row = nc.dram_tensor("bias_row", [1, F], dt.bfloat16, kind="ExternalInput")
    recip_row = nc.dram_tensor(
        "recip_row", [1, CHUNKS * 128], dt.bfloat16, kind="ExternalInput"
    )
    dinv_chk = nc.dram_tensor(
        "dinv_chk", [128, CHUNKS], dt.float32, kind="ExternalInput"
    )
    idx_lo = nc.dram_tensor("idx_lo", [128, 8 * blk_lo], dt.int16, kind="ExternalInput")
    idx_hi = nc.dram_tensor("idx_hi", [128, 8 * blk_hi], dt.int16, kind="ExternalInput")
    slots = nc.dram_tensor("slots", [128, n_segs], dt.bfloat16, kind="ExternalInput")
    out = nc.dram_tensor("out", [NPC, F], dt.bfloat16, kind="ExternalOutput")

    n_tiles_lo = (blk_lo + GB - 1) // GB
    n_tiles_hi = (blk_hi + GB - 1) // GB

    with tile.TileContext(nc) as tc, ExitStack() as top:
        cpool = top.enter_context(tc.tile_pool(name="const", bufs=1))
        # dummy SWDGE gather: pays the one-time Q7 ucode init (~10us) while
        # the idx/const DMAs stream in; idx tile built by DVE memset (no DMA)
        warm_idx = cpool.tile([128, 8], dt.int16)
        nc.vector.memset(warm_idx[:], 0)
        warm_g = cpool.tile([128, 1, F], dt.float8e3)
        nc.gpsimd.dma_gather(
            warm_g[:], xp[0:SPLIT, :], warm_idx[:], 128, 128, F,
            single_packet=False, queue_num=0,
        )
        ilo_s = cpool.tile([128, 8 * blk_lo], dt.int16)
        ihi_s = cpool.tile([128, 8 * blk_hi], dt.int16)
        head = 8 * GB * 6
        h_lo, h_hi = min(head, 8 * blk_lo), min(head, 8 * blk_hi)
        nc.sync.dma_start(out=ilo_s[:, 0:h_lo], in_=idx_lo[:, 0:h_lo])
        nc.sync.dma_start(out=ihi_s[:, 0:h_hi], in_=idx_hi[:, 0:h_hi])
        wt_s = cpool.tile([128, 2, F], dt.bfloat16)
        nc.sync.dma_start(out=wt_s[:, 0, :], in_=wT[0])
        nc.sync.dma_start(out=wt_s[:, 1, :], in_=wT[1])
        iota_s = cpool.tile([128, S_BATCH, 128], dt.bfloat16)
        nc.sync.dma_start(out=iota_s[:], in_=iota8[:])
        id_s = cpool.tile([128, 128], dt.bfloat16)
        nc.sync.dma_start(out=id_s[:], in_=ident[:])
        brow_s = cpool.tile([1, F], dt.bfloat16)
        nc.sync.dma_start(out=brow_s[:], in_=bias_row[:])
        rrow_s = cpool.tile([1, CHUNKS * 128], dt.bfloat16)
        nc.sync.dma_start(out=rrow_s[:], in_=recip_row[:])
        dvc_s = cpool.tile([128, CHUNKS], dt.float32)
        nc.sync.dma_start(out=dvc_s[:], in_=dinv_chk[:])
        slt_s = cpool.tile([128, n_segs], dt.bfloat16)
        nc.sync.dma_start(out=slt_s[:], in_=slots[:])
        if h_lo < 8 * blk_lo:
            nc.sync.dma_start(out=ilo_s[:, h_lo:], in_=idx_lo[:, h_lo:])
        if h_hi < 8 * blk_hi:
            nc.sync.dma_start(out=ihi_s[:, h_hi:], in_=idx_hi[:, h_hi:])

        with ExitStack() as p2:
            glo_pool = p2.enter_context(tc.tile_pool(name="glo", bufs=12))
            ghi_pool = p2.enter_context(tc.tile_pool(name="ghi", bufs=10))
            sfpool = p2.enter_context(tc.tile_pool(name="gself", bufs=3))
            spool = p2.enter_context(tc.tile_pool(name="sel", bufs=4))
            apool = p2.enter_context(tc.tile_pool(name="aggt", bufs=3))
            opool = p2.enter_context(tc.tile_pool(name="ostg", bufs=2))
            pa_pool = p2.enter_context(tc.tile_pool(name="pa", bufs=2, space="PSUM"))
            pb_pool = p2.enter_context(tc.tile_pool(name="pb", bufs=2, space="PSUM"))
            po_pool = p2.enter_context(tc.tile_pool(name="po", bufs=2, space="PSUM"))

            gq = [0]  # round-robin queue counter

            tiles = {}  # (w, t) -> (tile, nb)

            def ensure(w, t):
                if (w, t) in tiles:
                    return tiles[(w, t)]
                pool, blk_w, idx_s, base, nrows = (
                    (glo_pool, blk_lo, ilo_s, 0, SPLIT)
                    if w == 0
                    else (ghi_pool, blk_hi, ihi_s, SPLIT, N_NODES - SPLIT)
                )
                b0 = t * GB
                nb = min(GB, blk_w - b0)
                G = pool.tile([128, nb, F], dt.float8e3, tag=f"G{w}")
                nc.gpsimd.dma_gather(
                    G[:],
                    xp[base : base + nrows, :],
                    idx_s[:, 8 * b0 : 8 * (b0 + nb)],
                    128 * nb,
                    128 * nb,
                    F,
                    single_packet=False,
                    queue_num=gq[0] % N_QUEUES,
                )
                gq[0] += 1
                tiles[(w, t)] = (G, nb)
                return tiles[(w, t)]

            seg_idx = 0
            ob = None
            ob_base = 0
            og = 0
            for k in range(CHUNKS):
                if ob is None:
                    og = min(OUT_GRP, CHUNKS - k)
                    ob = opool.tile([128, og, F], dt.bfloat16, tag="ob")
                    ob_base = k

                # segments of this chunk, in consumption order
                ksegs = [(0,) + s for s in segs_lo[k]] + [
                    (1,) + s for s in segs_hi[k]
                ]
                # make sure needed gather tiles (plus one lookahead) exist
                for w, blk, p0, p1, off in ksegs:
                    ensure(w, blk // GB)
                last_lo = max((s[0] // GB for s in segs_lo[k]), default=-1)
                last_hi = max((s[0] // GB for s in segs_hi[k]), default=-1)
                for t in range(last_lo + 1, min(last_lo + 5, n_tiles_lo)):
                    ensure(0, t)
                for t in range(last_hi + 1, min(last_hi + 4, n_tiles_hi)):
                    ensure(1, t)

                # ---- self-loop rows: contiguous HWDGE load ----------------
                gs = sfpool.tile([128, F], dt.bfloat16, tag="gs")
                nc.sync.dma_start(out=gs[:], in_=xself[128 * k : 128 * (k + 1), :])

                # ---- one-hot S builds, batched ----------------------------
                s_tiles = []
                for s0 in range(0, len(ksegs), S_BATCH):
                    sb = min(S_BATCH, len(ksegs) - s0)
                    S = spool.tile([128, sb, 128], dt.bfloat16, tag="S")
                    nc.vector.tensor_tensor(
                        out=S[:],
                        in0=slt_s[
                            :, seg_idx + s0 : seg_idx + s0 + sb
                        ].to_broadcast([128, sb, 128]),
                        in1=iota_s[:, 0:sb, :],
                        op=mybir.AluOpType.is_equal,
                    )
                    s_tiles.append((s0, sb, S))

                # ---- segment-sum matmuls (f-major psum) -------------------
                psa = pa_pool.tile([128, 512], dt.float32)  # full bank
                psb = pb_pool.tile([128, 512], dt.float32)  # full bank
                for s0, sb, S in s_tiles:
                    for j in range(sb):
                        w, blk, p0, p1, off = ksegs[s0 + j]
                        G, nb = tiles[(w, blk // GB)]
                        bloc = blk - (blk // GB) * GB
                        first = s0 + j == 0
                        nc.tensor.matmul(
                            out=psa[:, 0:128],
                            lhsT=G[:, bloc, 0:128],
                            rhs=S[:, j, :],
                            start=first,
                            stop=False,
                        )
                        nc.tensor.matmul(
                            out=psb[:, 0:128],
                            lhsT=G[:, bloc, 128:256],
                            rhs=S[:, j, :],
                            start=first,
                            stop=False,
                        )
                # self-loop contribution closes the accumulation
                nc.tensor.matmul(
                    out=psa[:, 0:128],
                    lhsT=gs[:, 0:128],
                    rhs=id_s[:],
                    start=not ksegs,
                    stop=True,
                )
                nc.tensor.matmul(
                    out=psb[:, 0:128],
                    lhsT=gs[:, 128:256],
                    rhs=id_s[:],
                    start=not ksegs,
                    stop=True,
                )
                seg_idx += len(ksegs)

                # ---- psum -> sbuf (bf16) ----------------------------------
                at = apool.tile([128, 2, 128], dt.bfloat16, tag="at")
                nc.scalar.activation(
                    out=at[:, 0, :],
                    in_=psa[:, 0:128],
                    func=mybir.ActivationFunctionType.Copy,
                )
                nc.scalar.activation(
                    out=at[:, 1, :],
                    in_=psb[:, 0:128],
                    func=mybir.ActivationFunctionType.Copy,
                )

                # ---- projection + in-psum bias ----------------------------
                pot = po_pool.tile([128, 512], dt.float32)  # full bank
                po = pot[:, 0:F]
                nc.tensor.matmul(
                    out=po[:],
                    lhsT=at[:, 0, :],
                    rhs=wt_s[:, 0, :],
                    start=True,
                    stop=False,
                )
                nc.tensor.matmul(
                    out=po[:],
                    lhsT=at[:, 1, :],
                    rhs=wt_s[:, 1, :],
                    start=False,
                    stop=False,
                )
                nc.tensor.matmul(
                    out=po[:],
                    lhsT=rrow_s[:, 128 * k : 128 * (k + 1)],
                    rhs=brow_s[:],
                    start=False,
                    stop=True,
                )

                # ---- fused epilogue: relu(dinv * po) ----------------------
                nc.scalar.activation(
                    out=ob[:, k - ob_base, :],
                    in_=po[:],
                    func=mybir.ActivationFunctionType.Relu,
                    scale=dvc_s[:, k : k + 1],
                )

                if k - ob_base + 1 == og:
                    r0 = ob_base * 128
                    rw = og * 128
                    if r0 + rw <= NPC:
                        dst = out[r0 : r0 + rw, :].rearrange("(t p) f -> p t f", p=128)
                        nc.sync.dma_start(out=dst, in_=ob[:])
                    else:
                        # tail group: full chunks + one partial (106 rows)
                        full = (NPC - r0) // 128
                        if full:
                            dst = out[r0 : r0 + full * 128, :].rearrange(
                                "(t p) f -> p t f", p=128
                            )
                            nc.sync.dma_start(out=dst, in_=ob[:, :full, :])
                        rem = NPC - r0 - full * 128
                        if rem:
                            nc.sync.dma_start(
                                out=out[r0 + full * 128 : NPC, :],
                                in_=ob[:rem, full, :],
                            )
                    ob = None

    nc.compile()
    return nc


def _prep(x, edge_index, W, b):
    """Host-side sharding/layout. Returns (run_lo, run_hi, common, per_core)."""
    src = np.asarray(edge_index[0], dtype=np.int64)
    dst = np.asarray(edge_index[1], dtype=np.int64)
    deg = np.bincount(src, minlength=N_NODES).astype(np.float32)
    dinv = deg**-0.5

    # pre-scaled node features, e3m4-quantized at a global scale QS —
    # the only tensor the device gathers (256B rows halve gather DMA)
    xpf = np.asarray(x, dtype=np.float32) * dinv[:, None]
    xp = (xpf * QS).astype(FP8)

    # bucket NON-SELF edges by (core, chunk, lo/hi window); dst-sorted runs
    core = src // NPC
    src_local = src - core * NPC
    chunk = src_local >> 7
    slot = src_local & 127
    is_hi = (dst >= SPLIT).astype(np.int64)
    key = (core * CHUNKS + chunk) * 2 + is_hi
    order = np.lexsort((dst, key))
    key_s = key[order]
    dst_s = dst[order]
    slot_s = slot[order]

    nseg = N_CORES * CHUNKS * 2
    counts = np.bincount(key_s, minlength=nseg).reshape(N_CORES, CHUNKS, 2)
    seg_end = np.cumsum(counts.reshape(-1))
    seg_start = seg_end - counts.reshape(-1)

    run_lo = counts[:, :, 0].max(axis=0).astype(np.int64)  # [CHUNKS]
    run_hi = counts[:, :, 1].max(axis=0).astype(np.int64)

    # common (replicated) tensors
    wT = np.ascontiguousarray(np.asarray(W, dtype=np.float32).T).astype(BF16)
    wt_in = np.stack([wT[:128], wT[128:]])  # [2,128,F]
    iota_t = np.tile(
        np.arange(128, dtype=np.float32)[None, None, :], (128, S_BATCH, 1)
    ).astype(BF16)
    ident = np.eye(128, dtype=np.float32).astype(BF16)
    bias_row = np.asarray(b, dtype=np.float32)[None, :].astype(BF16)
    common = dict(xp=xp, wt=wt_in, iota8=iota_t, ident=ident, bias_row=bias_row)

    off_lo = np.concatenate([[0], np.cumsum(run_lo)])
    off_hi = np.concatenate([[0], np.cumsum(run_hi)])
    tot_lo, tot_hi = int(off_lo[-1]), int(off_hi[-1])
    segs_lo = _segments(run_lo, tot_lo)
    segs_hi = _segments(run_hi, tot_hi)
    n_segs = sum(len(segs_lo[k]) + len(segs_hi[k]) for k in range(CHUNKS))

    per_core = []
    for c in range(N_CORES):
        # packed idx streams
        pos_lo = []
        val_lo = []
        pos_hi = []
        val_hi = []
        slt = np.full((128, n_segs), 200.0, dtype=np.float32)
        si = 0
        for k in range(CHUNKS):
            s = (c * CHUNKS + k) * 2
            a0, a1 = seg_start[s], seg_end[s]
            b0, b1 = seg_start[s + 1], seg_end[s + 1]
            nlo, nhi = a1 - a0, b1 - b0
            pos_lo.append(off_lo[k] + np.arange(nlo))
            val_lo.append(dst_s[a0:a1])
            pos_hi.append(off_hi[k] + np.arange(nhi))
            val_hi.append(dst_s[b0:b1] - SPLIT)
            # slot columns per segment (consumption order: lo then hi)
            for w, (segs, q0, cnt, slots_v) in enumerate(
                (
                    (segs_lo[k], off_lo[k], nlo, slot_s[a0:a1]),
                    (segs_hi[k], off_hi[k], nhi, slot_s[b0:b1]),
                )
            ):
                for blk, p0, p1, off in segs:
                    # stream positions [off, off + (p1-p0)) map to partitions
                    # [p0, p1); valid rows are those with off+j < q0+cnt
                    j0 = off - q0  # position within this chunk's run
                    nvalid = max(0, min(cnt - j0, p1 - p0))
                    if nvalid > 0:
                        slt[p0 : p0 + nvalid, si] = slots_v[j0 : j0 + nvalid]
                    si += 1
        assert si == n_segs
        ilo = _pack_idx_stream(
            (np.concatenate(pos_lo), np.concatenate(val_lo)), tot_lo
        )
        ihi = _pack_idx_stream(
            (np.concatenate(pos_hi), np.concatenate(val_hi)), tot_hi
        )

        # per-core self rows (padded to 6272 with zeros)
        xself = np.zeros((NSELF, F), dtype=BF16)
        xself[:NPC] = (xpf[c * NPC : (c + 1) * NPC] * QS).astype(BF16)

        # dinv per (slot, chunk) for the epilogue scale
        nchk = np.arange(128)[:, None] + 128 * np.arange(CHUNKS)[None, :] + c * NPC
        dvc = np.where(
            nchk - c * NPC < NPC, dinv[np.minimum(nchk, N_NODES - 1)], 1.0
        ).astype(np.float32) / QS

        # sqrt(deg) per chunk-slot for the in-psum bias outer product
        rr = np.ones(CHUNKS * 128, dtype=np.float32)
        valid = np.arange(CHUNKS * 128) < NPC
        rr[valid] = QS * np.sqrt(deg[c * NPC : (c + 1) * NPC])
        recip_row = rr[None, :].astype(BF16)

        per_core.append(
            dict(
                idx_lo=ilo,
                idx_hi=ihi,
                slots=slt.astype(BF16),
                dinv_chk=np.ascontiguousarray(dvc),
                recip_row=recip_row,
                xself=xself,
            )
        )
    return run_lo, run_hi, common, per_core


def _install_ntff_hook():
    """The agent image's antenv lacks axon_hooks; recreate it so
    run_bass_kernel_spmd(trace=True) can profile via the axon .so."""
    import types

    if "antenv.axon_hooks" in sys.modules:
        return
    mod = types.ModuleType("antenv.axon_hooks")
    state = {}
    mod.set_axon_ntff_profile_hook = lambda h: state.__setitem__("h", h)
    mod.get_axon_ntff_profile_hook = lambda: state.get("h")
    sys.modules["antenv.axon_hooks"] = mod
    try:
        import antenv

        antenv.axon_hooks = mod
    except Exception:
        pass
    try:
        if "/root/.axon_site" not in sys.path:
            sys.path.insert(0, "/root/.axon_site")
        from trn_agent_boot.trn_boot import _ntff_profile_via_ctypes

        mod.set_axon_ntff_profile_hook(
            _ntff_profile_via_ctypes("/opt/axon/libaxon_pjrt.so")
        )
    except Exception:
        pass


_CACHE = {}


def kernel(x, edge_index, W, b, trace=False):
    if trace:
        _install_ntff_hook()
    run_lo, run_hi, common, per_core = _prep(x, edge_index, W, b)
    key = (tuple(run_lo), tuple(run_hi))
    if key not in _CACHE:
        _CACHE[key] = _build_program(run_lo, run_hi)
    nc = _CACHE[key]

    in_maps = []
    for c in range(N_CORES):
        m = dict(common)
        m.update(per_core[c])
        in_maps.append(m)

    res = run_bass_kernel_spmd(
        nc, in_maps, core_ids=list(range(N_CORES)), trace=trace
    )
    out = np.concatenate([r["out"] for r in res.results], axis=0)
    if trace:
        kernel.last_exec_ns = res.exec_time_ns
        kernel.last_profile = res.profile_json
    return out.astype(np.float32)
